# revision 50
# baseline (speedup 1.0000x reference)
"""Trainium2 Bass kernel for nn_EncoderTransformer_61194694033513.

Data-parallel over batch B=16 across 8 NeuronCores (2 batch elems per core).
Per core the whole forward runs out of SBUF with activations feature-major
HT[e, tok] in fp16 (one merged [128, 2N] tile per stream so elementwise ops
cover both feature halves with a single 3D-AP instruction); matmul
accumulation is fp32 in PSUM. Attention is flash-style (S^T tiles of
[128 keys x 512 queries], relu, accumulated into O^T) so the [N,N] matrix is
never materialized.

The two batch streams are software-pipelined: emission order is the Tile
scheduler's priority order, and each stream's LayerNorm chain (PE ones-reduce
stats -> partition-0 row math -> rstd/mu rows bounced through DRAM and
broadcast back by partition-stride-0 DMAs on the two HW DGE queues -> 2-op
fp16 apply) drains while the PE runs the other stream's attention/qkv/mlp
matmuls. LN1 stats ride chunk-wise inside the other stream's attention block.
When g==1 and be==0 (the reference's values) the apply skips the affine op
(FAST_LN); a general 3-op path is compiled otherwise. The readout runs in
fp16 (fp32 matmuls are quarter-rate on the PE).
"""

import sys

import numpy as np

for _p in (
    "/opt/trn_rl_repo",
    "/opt/pypackages",
    "/root/.axon_site",
    "/root/.axon_site/_ro/trn_rl_repo",
    "/root/.axon_site/_ro/pypackages",
):
    if _p not in sys.path:
        sys.path.append(_p)

import ml_dtypes  # noqa: E402

import concourse.bass as bass  # noqa: E402
import concourse.bacc as bacc  # noqa: E402
import concourse.mybir as mybir  # noqa: E402
from concourse import tile  # noqa: E402
from concourse.bass_utils import run_bass_kernel_spmd  # noqa: E402

B, N, D, E, L = 16, 2048, 128, 256, 3
NCORES = 8
BL = B // NCORES  # batch elems per core
P = 128
EC = E // P  # feature-dim partition chunks (2)
CH = N // 512  # 512-wide token chunks (4)
JT = N // P  # key tiles (16)
EPS = 1e-5
F32 = mybir.dt.float32
F16 = mybir.dt.float16
NPF16 = np.float16
AF = mybir.ActivationFunctionType
OP = mybir.AluOpType

# relu engine per j2 slot within a chunk (Pool can't read PSUM, so only
# Act / DVE take the relu of the [128,1024] fp32 S tiles)
RELU_ENG = ["D", "A", "D", "A", "D", "A", "D", "A"]

_CACHE = {}


def _build(FAST_LN=True):
    nc = bacc.Bacc("TRN2", target_bir_lowering=False, debug=False, num_devices=NCORES)

    d_xsT = nc.declare_dram_parameter("xsT", [BL, P, N], F16, isOutput=False)
    d_Win = nc.declare_dram_parameter("Win", [D, E], F16, isOutput=False)
    d_W = {
        nm: nc.declare_dram_parameter(nm, [L, E, E], F16, isOutput=False)
        for nm in ("Wq", "Wk", "Wv", "W1", "W2")
    }
    d_WoutT = nc.declare_dram_parameter("WoutT", [E, N], F16, isOutput=False)
    d_colpack = nc.declare_dram_parameter("colpack", [P, 2 + 6 * L * EC], F32, isOutput=False)
    d_bout = nc.declare_dram_parameter("b_out", [1, 1], F32, isOutput=False)
    d_out = nc.declare_dram_parameter("out", [BL, 1], F32, isOutput=True)

    with tile.TileContext(nc) as tc:
        from contextlib import ExitStack

        with ExitStack() as ctx:
            cpool = ctx.enter_context(tc.tile_pool(name="const", bufs=1))
            hpool = ctx.enter_context(tc.tile_pool(name="acts", bufs=1))
            xs_pool = ctx.enter_context(tc.tile_pool(name="xs", bufs=2))
            spool = ctx.enter_context(tc.tile_pool(name="srelu", bufs=8))
            sqpool = ctx.enter_context(tc.tile_pool(name="sqp", bufs=6))
            apool = ctx.enter_context(tc.tile_pool(name="mlpa", bufs=4))
            tpool = ctx.enter_context(tc.tile_pool(name="t1p", bufs=4))
            bcpool = ctx.enter_context(tc.tile_pool(name="bc", bufs=6))
            ropool = ctx.enter_context(tc.tile_pool(name="ro", bufs=1))
            rfpool = ctx.enter_context(tc.tile_pool(name="rf", bufs=3))
            # DRAM bounce pool for LN row broadcasts (SBUF sources can't
            # take partition-stride-0 DMAs; DRAM sources can)
            drpool = ctx.enter_context(tc.tile_pool(name="drb", bufs=6, space="DRAM"))

            PS = bass.MemorySpace.PSUM
            ps_s = ctx.enter_context(tc.tile_pool(name="ps_s", bufs=2, space=PS))
            ps_o = ctx.enter_context(tc.tile_pool(name="ps_o", bufs=2, space=PS))
            ps_mm = ctx.enter_context(tc.tile_pool(name="ps_mm", bufs=2, space=PS))

            # ---- prologue DMAs, ordered by first use; xs in chunks so the
            # first input-proj matmul starts after 128KB, not 512KB --------
            xs_tiles = []
            xt = xs_pool.tile([P, N], F16, name="xst0", tag="xst0")
            nc.sync.dma_start(xt[:, 0:512], d_xsT[0, :, 0:512])
            xs_tiles.append(xt)
            win_sb = cpool.tile([P, E], F16, name="win", tag="win")
            nc.sync.dma_start(win_sb[:], d_Win[:])
            colpack = cpool.tile([P, 2 + 6 * L * EC], F32, name="colpack", tag="colpack")
            nc.sync.dma_start(colpack[:], d_colpack[:])
            for c in range(1, CH):
                nc.sync.dma_start(xt[:, c * 512 : (c + 1) * 512], d_xsT[0, :, c * 512 : (c + 1) * 512])
            xt1 = xs_pool.tile([P, N], F16, name="xst1", tag="xst1")
            nc.sync.dma_start(xt1[:], d_xsT[1])
            xs_tiles.append(xt1)

            # per-layer weight tiles so layer-0 deps resolve early
            w_sb = {nm: [[None] * EC for _ in range(L)] for nm in ("Wq", "Wk", "Wv", "W1", "W2")}

            def load_layer_weights(l, names):
                for nm in names:
                    for ec in range(EC):
                        t = cpool.tile([P, E], F16, name=f"{nm}{l}{ec}", tag=f"{nm}{l}{ec}")
                        nc.sync.dma_start(t[:], d_W[nm][l, ec * P : (ec + 1) * P, :])
                        w_sb[nm][l][ec] = t

            load_layer_weights(0, ("Wk", "Wq", "Wv"))
            load_layer_weights(0, ("W1", "W2"))
            for l in range(1, L):
                load_layer_weights(l, ("Wk", "Wq", "Wv", "W1", "W2"))

            binp_sb = colpack[:, 0:EC]

            def col_views(base):
                return [
                    colpack[:, 2 + base * L * EC + l * EC : 2 + base * L * EC + (l + 1) * EC]
                    for l in range(L)
                ]

            bm1_sb = col_views(0)
            bm2_sb = col_views(1)
            be1_sb = col_views(2)
            be2_sb = col_views(3)
            g1_sb = col_views(4)
            g2_sb = col_views(5)
            bout_sb = cpool.tile([1, 1], F32, name="bout", tag="bout")
            nc.sync.dma_start(bout_sb[:], d_bout[:])
            # merged [P, 2N] WoutT so readout prods run as one 3D op per c2
            wout_sb = cpool.tile([P, 2 * N], F16, name="wout", tag="wout")
            for ec in range(EC):
                nc.sync.dma_start(
                    wout_sb[:, ec * N : (ec + 1) * N], d_WoutT[ec * P : (ec + 1) * P, :]
                )

            ones_kb = cpool.tile([P, 1], F16, name="ones_kb", tag="ones_kb")
            nc.vector.memset(ones_kb[:], 1.0)
            eps1 = cpool.tile([1, 1], F32, name="eps1", tag="eps1")
            nc.vector.memset(eps1[:], EPS)

            # LN row scratch, one set per stream. rowsB fp16 holds the
            # rstd / mu rows = the broadcast DMA source.
            rowsB = [cpool.tile([1, 2 * N], F16, name=f"rowsB{b}", tag=f"rowsB{b}") for b in range(BL)]

            # ---- persistent activations (fp16): H merged [P, 2N] per
            # stream (feature half ec lives at columns [ec*N, (ec+1)*N)) ----
            Hf = [hpool.tile([P, 2 * N], F16, name=f"Hf{b}", tag=f"Hf{b}") for b in range(BL)]
            qT = [[hpool.tile([P, N], F16, name=f"qT{b}{dc}", tag=f"qT{b}{dc}") for dc in range(EC)] for b in range(BL)]
            kT = [[hpool.tile([P, N], F16, name=f"kT{b}{dc}", tag=f"kT{b}{dc}") for dc in range(EC)] for b in range(BL)]
            v_sb = [hpool.tile([P, JT * E], F16, name=f"v{b}", tag=f"v{b}") for b in range(BL)]

            def Hsl(b, ec, sl):
                return Hf[b][:, ec * N + sl.start : ec * N + sl.stop]

            def H3(b, sl):
                # [128, EC, len] view of both feature halves of a token range
                return Hf[b][:].rearrange("p (e n) -> p e n", e=EC)[:, :, sl]

            def bc3(t, width):
                # [128, EC, width] broadcast of a [P, width] tile over halves
                return t[:].rearrange("p (a n) -> p a n", a=1).to_broadcast((P, EC, width))

            def ln_squares(b, c):
                cs = slice(c * 512, (c + 1) * 512)
                sq = sqpool.tile([P, 1024], F16, name="sq", tag="sq")
                sq3 = sq[:].rearrange("p (e n) -> p e n", e=EC)
                if c % 2 == 0:
                    nc.scalar.activation(sq3, H3(b, cs), AF.Square, scale=1.0 / 64)
                else:
                    nc.vector.scalar_tensor_tensor(
                        sq3, H3(b, cs), 1.0 / 4096.0, H3(b, cs),
                        op0=OP.mult, op1=OP.mult,
                    )
                return sq

            def ln_stats_chunk(b, c, sq=None, qpool=None):
                """PE ones-reduce stats + partition-0 row math for one
                512-token chunk of Hf[b]. When the attention ps_o pool is
                idle (bare LN phases) st_q borrows its slots so the mm ring
                only carries st_s."""
                rstd_row = rowsB[b][:, 0:N]
                mu_row = rowsB[b][:, N : 2 * N]
                cs = slice(c * 512, (c + 1) * 512)
                if sq is None:
                    sq = ln_squares(b, c)
                st_s = ps_mm.tile([1, 512], F32, name="st_s", tag="mm")
                nc.tensor.matmul(st_s[:], ones_kb[:], Hsl(b, 0, cs), start=True, stop=False)
                nc.tensor.matmul(st_s[:], ones_kb[:], Hsl(b, 1, cs), start=False, stop=True)
                pool = qpool if qpool is not None else ps_mm
                st_q = pool.tile([1, 512], F32, name="st_q", tag="mm" if qpool is None else "o")
                nc.tensor.matmul(st_q[:], ones_kb[:], sq[:, 0:512], start=True, stop=False)
                nc.tensor.matmul(st_q[:], ones_kb[:], sq[:, 512:1024], start=False, stop=True)
                # xsq = (s0/64)^2 ; mu = s0/E ; xr = stq - xsq/E ;
                # rstd = 1/sqrt(xr*4096/E + eps)
                rf = rfpool.tile([1, 512], F32, name="rf", tag="rf")
                xrc = rf[:, 0:512]
                nc.scalar.activation(xrc, st_s[:], AF.Square, scale=1.0 / 64)
                nc.scalar.activation(mu_row[:, cs], st_s[:], AF.Identity, scale=1.0 / E)
                nc.vector.scalar_tensor_tensor(
                    xrc, xrc, -1.0 / E, st_q[:], op0=OP.mult, op1=OP.add
                )
                nc.scalar.activation(
                    rstd_row[:, cs], xrc, AF.Abs_reciprocal_sqrt,
                    bias=eps1[:], scale=4096.0 / E,
                )

            def ln_finish(b, g_col, be_col):
                """Broadcast rstd / mu rows back over partitions via the DRAM
                bounce on the two HW DGE queues, then the fp16 apply: one 3D
                sub + one 3D mul per chunk-pair covering both feature halves
                (plus the affine op when not FAST_LN)."""
                rstd_row = rowsB[b][:, 0:N]
                mu_row = rowsB[b][:, N : 2 * N]
                for c2 in range(CH // 2):
                    cs2 = slice(c2 * 1024, (c2 + 1) * 1024)
                    drr = drpool.tile([1, 1024], F16, name="drr", tag="drr")
                    nc.sync.dma_start(drr[:], rstd_row[:, cs2])
                    drm = drpool.tile([1, 1024], F16, name="drm", tag="drm")
                    nc.scalar.dma_start(drm[:], mu_row[:, cs2])
                    rb = bcpool.tile([P, 1024], F16, name="rb", tag="rb")
                    nc.sync.dma_start(rb[:], drr[:].to_broadcast((P, 1024)))
                    mb = bcpool.tile([P, 1024], F16, name="mb", tag="mb")
                    nc.scalar.dma_start(mb[:], drm[:].to_broadcast((P, 1024)))
                    if FAST_LN:
                        t1 = tpool.tile([P, 2048], F16, name="t1", tag="t1")
                        t13 = t1[:].rearrange("p (e n) -> p e n", e=EC)
                        nc.vector.tensor_tensor(
                            t13, H3(b, cs2), bc3(mb, 1024), op=OP.subtract
                        )
                        nc.vector.tensor_tensor(
                            H3(b, cs2), t13, bc3(rb, 1024), op=OP.mult
                        )
                    else:
                        for pt in range(EC):
                            t1 = tpool.tile([P, 1024], F16, name="t1", tag="t1")
                            nc.vector.tensor_sub(t1[:], Hsl(b, pt, cs2), mb[:])
                            t2 = tpool.tile([P, 1024], F16, name="t2", tag="t2")
                            nc.vector.tensor_mul(t2[:], t1[:], rb[:])
                            nc.vector.tensor_scalar(
                                Hsl(b, pt, cs2), t2[:],
                                g_col[:, pt : pt + 1], be_col[:, pt : pt + 1],
                                op0=OP.mult, op1=OP.add,
                            )

            def layernorm(b, g_col, be_col):
                sqs = [ln_squares(b, c) for c in range(CH)]
                for c in range(CH):
                    ln_stats_chunk(b, c, sqs[c], qpool=ps_o)
                ln_finish(b, g_col, be_col)

            def input_proj(b):
                xs_t = xs_tiles[b]
                for ec in range(EC):
                    es = slice(ec * P, (ec + 1) * P)
                    for c in range(CH):
                        cs = slice(c * 512, (c + 1) * 512)
                        ps = ps_mm.tile([P, 512], F32, name="psin", tag="mm")
                        nc.tensor.matmul(ps[:], win_sb[:, es], xs_t[:, cs])
                        nc.vector.tensor_scalar_add(Hsl(b, ec, cs), ps[:], binp_sb[:, ec : ec + 1])

            def qkv(b, l):
                for w_name, dstT in (("Wk", kT[b]), ("Wq", qT[b])):
                    for dc in range(EC):
                        ds_ = slice(dc * P, (dc + 1) * P)
                        for c in range(CH):
                            cs = slice(c * 512, (c + 1) * 512)
                            ps = ps_mm.tile([P, 512], F32, name="psqk", tag="mm")
                            for ec in range(EC):
                                nc.tensor.matmul(
                                    ps[:],
                                    w_sb[w_name][l][ec][:, ds_],
                                    Hsl(b, ec, cs),
                                    start=(ec == 0),
                                    stop=(ec == EC - 1),
                                )
                            nc.scalar.copy(dstT[dc][:, cs], ps[:])
                for t in range(JT):
                    ps = ps_mm.tile([P, E], F32, name="psv", tag="mm")
                    for ec in range(EC):
                        nc.tensor.matmul(
                            ps[:],
                            Hsl(b, ec, slice(t * P, (t + 1) * P)),
                            w_sb["Wv"][l][ec][:],
                            start=(ec == 0),
                            stop=(ec == EC - 1),
                        )
                    if t % 2 == 0:
                        nc.scalar.copy(v_sb[b][:, t * E : (t + 1) * E], ps[:])
                    else:
                        nc.vector.tensor_copy(v_sb[b][:, t * E : (t + 1) * E], ps[:])

            def attention_chunk(b, c):
                cs = slice(c * 512, (c + 1) * 512)
                o_ps = [
                    ps_o.tile([P, 512], F32, name=f"o{oc}", tag="o")
                    for oc in range(EC)
                ]
                for j2 in range(JT // 2):
                    s_ps = ps_s.tile([P, 1024], F32, name="s_ps", tag="s")
                    for h in range(2):
                        j = 2 * j2 + h
                        hs = slice(h * 512, (h + 1) * 512)
                        for dc in range(EC):
                            nc.tensor.matmul(
                                s_ps[:, hs],
                                kT[b][dc][:, j * P : (j + 1) * P],
                                qT[b][dc][:, cs],
                                start=(dc == 0),
                                stop=(dc == EC - 1),
                            )
                    sr = spool.tile([P, 1024], F16, name="sr", tag="sr")
                    if RELU_ENG[j2] == "D":
                        nc.vector.tensor_relu(sr[:], s_ps[:])
                    else:
                        nc.scalar.activation(sr[:], s_ps[:], AF.Relu)
                    for h in range(2):
                        j = 2 * j2 + h
                        hs = slice(h * 512, (h + 1) * 512)
                        for oc in range(EC):
                            nc.tensor.matmul(
                                o_ps[oc][:],
                                v_sb[b][:, j * E + oc * P : j * E + (oc + 1) * P],
                                sr[:, hs],
                                start=(j == 0),
                                stop=(j == JT - 1),
                            )
                for oc in range(EC):
                    nc.vector.tensor_add(Hsl(b, oc, cs), Hsl(b, oc, cs), o_ps[oc][:])

            def attention(b):
                for c in range(CH):
                    attention_chunk(b, c)

            def mlp_chunk(b, l, c):
                cs = slice(c * 512, (c + 1) * 512)
                a_t = []
                for mc in range(EC):
                    ms = slice(mc * P, (mc + 1) * P)
                    ps = ps_mm.tile([P, 512], F32, name="psa", tag="mm")
                    for ec in range(EC):
                        nc.tensor.matmul(
                            ps[:],
                            w_sb["W1"][l][ec][:, ms],
                            Hsl(b, ec, cs),
                            start=(ec == 0),
                            stop=(ec == EC - 1),
                        )
                    a = apool.tile([P, 512], F16, name="a", tag="a")
                    nc.scalar.activation(
                        a[:], ps[:], AF.Relu, bias=bm1_sb[l][:, mc : mc + 1]
                    )
                    a_t.append(a)
                for oc in range(EC):
                    os_ = slice(oc * P, (oc + 1) * P)
                    ps = ps_mm.tile([P, 512], F32, name="psm", tag="mm")
                    for mc in range(EC):
                        nc.tensor.matmul(
                            ps[:],
                            w_sb["W2"][l][mc][:, os_],
                            a_t[mc][:],
                            start=(mc == 0),
                            stop=(mc == EC - 1),
                        )
                    nc.vector.scalar_tensor_tensor(
                        Hsl(b, oc, cs),
                        ps[:],
                        bm2_sb[l][:, oc : oc + 1],
                        Hsl(b, oc, cs),
                        op0=OP.add,
                        op1=OP.add,
                    )

            def mlp(b, l):
                for c in range(CH):
                    mlp_chunk(b, l, c)

            def readout(b):
                # prods = H (x) WoutT elementwise fp16, one 3D op per c2
                # pair; token-partials accumulate into one [1,512] PSUM row.
                prods = ropool.tile([P, 2 * N], F16, name="ros", tag="ros")
                wout3 = wout_sb[:].rearrange("p (e n) -> p e n", e=EC)
                st = ps_o.tile([1, 512], F32, name="psro", tag="o")
                nmm = 2 * CH
                i = 0
                for c2 in range(CH // 2):
                    cs2 = slice(c2 * 1024, (c2 + 1) * 1024)
                    prods3 = prods[:].rearrange("p (e n) -> p e n", e=EC)[:, :, cs2]
                    nc.vector.tensor_tensor(
                        prods3, H3(b, cs2), wout3[:, :, cs2], op=OP.mult
                    )
                    for cc in range(2):
                        cs = slice((2 * c2 + cc) * 512, (2 * c2 + cc + 1) * 512)
                        for ec in range(EC):
                            nc.tensor.matmul(
                                st[:], ones_kb[:],
                                prods[:, ec * N + cs.start : ec * N + cs.stop],
                                start=(i == 0), stop=(i == nmm - 1),
                            )
                            i += 1
                rsc = cpool.tile([1, 1], F32, name=f"rsc{b}", tag=f"rsc{b}")
                nc.vector.reduce_sum(rsc[:], st[:], axis=mybir.AxisListType.X)
                ob = cpool.tile([1, 1], F32, name=f"ob{b}", tag=f"ob{b}")
                nc.scalar.activation(ob[:], rsc[:], AF.Identity, bias=bout_sb[:])
                nc.sync.dma_start(d_out[b : b + 1, :], ob[:])

            # ---- software-pipelined emission ------------------------------
            input_proj(0)
            input_proj(1)
            qkv(0, 0)
            for l in range(L):
                attention(0)
                qkv(1, l)
                # LN1(b0) stats ride inside attention(b1): stats chunk c
                # after attention chunk c, so PE never waits on squares
                for c in range(CH):
                    attention_chunk(1, c)
                    ln_stats_chunk(0, c)
                ln_finish(0, g1_sb[l], be1_sb[l])
                mlp(0, l)
                layernorm(1, g1_sb[l], be1_sb[l])
                layernorm(0, g2_sb[l], be2_sb[l])
                mlp(1, l)
                if l + 1 < L:
                    qkv(0, l + 1)
                    layernorm(1, g2_sb[l], be2_sb[l])
                else:
                    readout(0)
                    layernorm(1, g2_sb[l], be2_sb[l])
                    readout(1)

    nc.compile()
    return nc


def _prep_inputs(inputs):
    f = lambda x: np.asarray(x, np.float32)
    bf = lambda x: np.ascontiguousarray(np.asarray(x, np.float32).astype(NPF16))
    xs = f(inputs["xs"])
    xsT = np.ascontiguousarray(xs.transpose(0, 2, 1)).astype(NPF16)  # [B, D, N]
    WoutT = np.ascontiguousarray(f(inputs["Wout"]).reshape(N, E).T)  # [E, N]

    def cols(v, per_l):
        v = f(v)
        if per_l:
            return np.ascontiguousarray(v.reshape(L, EC, P).transpose(0, 2, 1))
        return np.ascontiguousarray(v.reshape(EC, P).T)

    common = {
        "Win": bf(inputs["Win"]),
        "Wq": bf(inputs["Wq"]),
        "Wk": bf(inputs["Wk"]),
        "Wv": bf(inputs["Wv"]),
        "W1": bf(inputs["W1"]),
        "W2": bf(inputs["W2"]),
        "WoutT": WoutT.astype(NPF16),
        "colpack": np.concatenate(
            [cols(inputs["b_in"], False)]
            + [
                cols(inputs[k], True).transpose(1, 0, 2).reshape(P, L * EC)
                for k in ("bm1", "bm2", "be1", "be2", "g1", "g2")
            ],
            axis=1,
        ),
        "b_out": f(inputs["b_out"]).reshape(1, 1),
    }
    in_maps = []
    for c in range(NCORES):
        m = dict(common)
        m["xsT"] = np.ascontiguousarray(xsT[c * BL : (c + 1) * BL])
        in_maps.append(m)
    return in_maps


def get_program(fast_ln=True):
    key = f"nc{fast_ln}"
    if key not in _CACHE:
        _CACHE[key] = _build(FAST_LN=fast_ln)
    return _CACHE[key]


def kernel(**inputs) -> np.ndarray:
    fast = (
        np.all(np.asarray(inputs["g1"]) == 1.0)
        and np.all(np.asarray(inputs["g2"]) == 1.0)
        and np.all(np.asarray(inputs["be1"]) == 0.0)
        and np.all(np.asarray(inputs["be2"]) == 0.0)
    )
    nc = get_program(fast_ln=bool(fast))
    in_maps = _prep_inputs(inputs)
    res = run_bass_kernel_spmd(nc, in_maps, list(range(NCORES)))
    out = np.concatenate([res.results[c]["out"] for c in range(NCORES)], axis=0)
    return out.astype(np.float32)


# revision 51
# speedup vs baseline: 1.0453x; 1.0453x over previous
"""Trainium2 Bass kernel for nn_EncoderTransformer_61194694033513.

Data-parallel over batch B=16 across 8 NeuronCores (2 batch elems per core).
Per core the whole forward runs out of SBUF with activations feature-major
HT[e, tok] in fp16 (one merged [128, 2N] tile per stream so elementwise ops
cover both feature halves with a single 3D-AP instruction); matmul
accumulation is fp32 in PSUM. Attention is flash-style (S^T tiles of
[128 keys x 512 queries], relu, accumulated into O^T) so the [N,N] matrix is
never materialized.

The two batch streams are software-pipelined: emission order is the Tile
scheduler's priority order, and each stream's LayerNorm chain (PE ones-reduce
stats -> partition-0 row math -> rstd/mu rows bounced through DRAM and
broadcast back by partition-stride-0 DMAs on the two HW DGE queues -> 2-op
fp16 apply) drains while the PE runs the other stream's attention/qkv/mlp
matmuls. LN1 stats ride chunk-wise inside the other stream's attention block.
When g==1 and be==0 (the reference's values) the apply skips the affine op
(FAST_LN); a general 3-op path is compiled otherwise. The readout runs in
fp16 (fp32 matmuls are quarter-rate on the PE).
"""

import sys

import numpy as np

for _p in (
    "/opt/trn_rl_repo",
    "/opt/pypackages",
    "/root/.axon_site",
    "/root/.axon_site/_ro/trn_rl_repo",
    "/root/.axon_site/_ro/pypackages",
):
    if _p not in sys.path:
        sys.path.append(_p)

import ml_dtypes  # noqa: E402

import concourse.bass as bass  # noqa: E402
import concourse.bacc as bacc  # noqa: E402
import concourse.mybir as mybir  # noqa: E402
from concourse import tile  # noqa: E402
from concourse.bass_utils import run_bass_kernel_spmd  # noqa: E402

B, N, D, E, L = 16, 2048, 128, 256, 3
NCORES = 8
BL = B // NCORES  # batch elems per core
P = 128
EC = E // P  # feature-dim partition chunks (2)
CH = N // 512  # 512-wide token chunks (4)
JT = N // P  # key tiles (16)
EPS = 1e-5
F32 = mybir.dt.float32
F16 = mybir.dt.float16
NPF16 = np.float16
AF = mybir.ActivationFunctionType
OP = mybir.AluOpType

# relu engine per j2 slot within a chunk (Pool can't read PSUM, so only
# Act / DVE take the relu of the [128,1024] fp32 S tiles)
RELU_ENG = ["D", "A", "D", "A", "D", "A", "D", "A"]

_CACHE = {}


def _build(FAST_LN=True):
    nc = bacc.Bacc("TRN2", target_bir_lowering=False, debug=False, num_devices=NCORES)

    d_xsT = nc.declare_dram_parameter("xsT", [BL, P, N], F16, isOutput=False)
    d_Win = nc.declare_dram_parameter("Win", [D, E], F16, isOutput=False)
    d_W = {
        nm: nc.declare_dram_parameter(nm, [L, E, E], F16, isOutput=False)
        for nm in ("Wq", "Wk", "Wv", "W1", "W2")
    }
    d_WoutT = nc.declare_dram_parameter("WoutT", [E, N], F16, isOutput=False)
    d_colpack = nc.declare_dram_parameter("colpack", [P, 2 + 6 * L * EC], F32, isOutput=False)
    d_bout = nc.declare_dram_parameter("b_out", [1, 1], F32, isOutput=False)
    d_out = nc.declare_dram_parameter("out", [BL, 1], F32, isOutput=True)

    with tile.TileContext(nc) as tc:
        from contextlib import ExitStack

        with ExitStack() as ctx:
            cpool = ctx.enter_context(tc.tile_pool(name="const", bufs=1))
            hpool = ctx.enter_context(tc.tile_pool(name="acts", bufs=1))
            xs_pool = ctx.enter_context(tc.tile_pool(name="xs", bufs=2))
            spool = ctx.enter_context(tc.tile_pool(name="srelu", bufs=8))
            sqpool = ctx.enter_context(tc.tile_pool(name="sqp", bufs=6))
            apool = ctx.enter_context(tc.tile_pool(name="mlpa", bufs=4))
            tpool = ctx.enter_context(tc.tile_pool(name="t1p", bufs=4))
            bcpool = ctx.enter_context(tc.tile_pool(name="bc", bufs=6))
            ropool = ctx.enter_context(tc.tile_pool(name="ro", bufs=1))
            rfpool = ctx.enter_context(tc.tile_pool(name="rf", bufs=3))
            # DRAM bounce pool for LN row broadcasts (SBUF sources can't
            # take partition-stride-0 DMAs; DRAM sources can)
            drpool = ctx.enter_context(tc.tile_pool(name="drb", bufs=6, space="DRAM"))

            PS = bass.MemorySpace.PSUM
            ps_s = ctx.enter_context(tc.tile_pool(name="ps_s", bufs=2, space=PS))
            ps_o = ctx.enter_context(tc.tile_pool(name="ps_o", bufs=2, space=PS))
            ps_mm = ctx.enter_context(tc.tile_pool(name="ps_mm", bufs=2, space=PS))

            # ---- prologue DMAs, ordered by first use; xs in chunks so the
            # first input-proj matmul starts after 128KB, not 512KB --------
            xs_tiles = []
            xt = xs_pool.tile([P, N], F16, name="xst0", tag="xst0")
            nc.sync.dma_start(xt[:, 0:512], d_xsT[0, :, 0:512])
            xs_tiles.append(xt)
            win_sb = cpool.tile([P, E], F16, name="win", tag="win")
            nc.sync.dma_start(win_sb[:], d_Win[:])
            colpack = cpool.tile([P, 2 + 6 * L * EC], F32, name="colpack", tag="colpack")
            nc.sync.dma_start(colpack[:], d_colpack[:])
            for c in range(1, CH):
                nc.sync.dma_start(xt[:, c * 512 : (c + 1) * 512], d_xsT[0, :, c * 512 : (c + 1) * 512])
            xt1 = xs_pool.tile([P, N], F16, name="xst1", tag="xst1")
            nc.sync.dma_start(xt1[:], d_xsT[1])
            xs_tiles.append(xt1)

            # per-layer weight tiles so layer-0 deps resolve early
            w_sb = {nm: [[None] * EC for _ in range(L)] for nm in ("Wq", "Wk", "Wv", "W1", "W2")}

            def load_layer_weights(l, names):
                for nm in names:
                    for ec in range(EC):
                        t = cpool.tile([P, E], F16, name=f"{nm}{l}{ec}", tag=f"{nm}{l}{ec}")
                        nc.sync.dma_start(t[:], d_W[nm][l, ec * P : (ec + 1) * P, :])
                        w_sb[nm][l][ec] = t

            load_layer_weights(0, ("Wk", "Wq", "Wv"))
            load_layer_weights(0, ("W1", "W2"))
            for l in range(1, L):
                load_layer_weights(l, ("Wk", "Wq", "Wv", "W1", "W2"))

            binp_sb = colpack[:, 0:EC]

            def col_views(base):
                return [
                    colpack[:, 2 + base * L * EC + l * EC : 2 + base * L * EC + (l + 1) * EC]
                    for l in range(L)
                ]

            bm1_sb = col_views(0)
            bm2_sb = col_views(1)
            be1_sb = col_views(2)
            be2_sb = col_views(3)
            g1_sb = col_views(4)
            g2_sb = col_views(5)
            bout_sb = cpool.tile([1, 1], F32, name="bout", tag="bout")
            nc.sync.dma_start(bout_sb[:], d_bout[:])
            # merged [P, 2N] WoutT so readout prods run as one 3D op per c2
            wout_sb = cpool.tile([P, 2 * N], F16, name="wout", tag="wout")
            for ec in range(EC):
                nc.sync.dma_start(
                    wout_sb[:, ec * N : (ec + 1) * N], d_WoutT[ec * P : (ec + 1) * P, :]
                )

            ones_kb = cpool.tile([P, 1], F16, name="ones_kb", tag="ones_kb")
            nc.vector.memset(ones_kb[:], 1.0)
            eps1 = cpool.tile([1, 1], F32, name="eps1", tag="eps1")
            nc.vector.memset(eps1[:], EPS)

            # LN row scratch, one set per stream. rowsB fp16 holds the
            # rstd / mu rows = the broadcast DMA source.
            rowsB = [cpool.tile([1, 2 * N], F16, name=f"rowsB{b}", tag=f"rowsB{b}") for b in range(BL)]

            # ---- persistent activations (fp16): H merged [P, 2N] per
            # stream (feature half ec lives at columns [ec*N, (ec+1)*N)) ----
            Hf = [hpool.tile([P, 2 * N], F16, name=f"Hf{b}", tag=f"Hf{b}") for b in range(BL)]
            qT = [[hpool.tile([P, N], F16, name=f"qT{b}{dc}", tag=f"qT{b}{dc}") for dc in range(EC)] for b in range(BL)]
            kT = [[hpool.tile([P, N], F16, name=f"kT{b}{dc}", tag=f"kT{b}{dc}") for dc in range(EC)] for b in range(BL)]
            v_sb = [hpool.tile([P, JT * E], F16, name=f"v{b}", tag=f"v{b}") for b in range(BL)]

            def Hsl(b, ec, sl):
                return Hf[b][:, ec * N + sl.start : ec * N + sl.stop]

            def H3(b, sl):
                # [128, EC, len] view of both feature halves of a token range
                return Hf[b][:].rearrange("p (e n) -> p e n", e=EC)[:, :, sl]

            def bc3(t, width):
                # [128, EC, width] broadcast of a [P, width] tile over halves
                return t[:].rearrange("p (a n) -> p a n", a=1).to_broadcast((P, EC, width))

            def ln_squares(b, c):
                cs = slice(c * 512, (c + 1) * 512)
                sq = sqpool.tile([P, 1024], F16, name="sq", tag="sq")
                sq3 = sq[:].rearrange("p (e n) -> p e n", e=EC)
                if c % 2 == 0:
                    nc.scalar.activation(sq3, H3(b, cs), AF.Square, scale=1.0 / 64)
                else:
                    nc.vector.scalar_tensor_tensor(
                        sq3, H3(b, cs), 1.0 / 4096.0, H3(b, cs),
                        op0=OP.mult, op1=OP.mult,
                    )
                return sq

            def ln_stats_chunk(b, c, sq=None):
                """PE ones-reduce stats + partition-0 row math for one
                512-token chunk of Hf[b]."""
                rstd_row = rowsB[b][:, 0:N]
                mu_row = rowsB[b][:, N : 2 * N]
                cs = slice(c * 512, (c + 1) * 512)
                if sq is None:
                    sq = ln_squares(b, c)
                st_s = ps_mm.tile([1, 512], F32, name="st_s", tag="mm")
                nc.tensor.matmul(st_s[:], ones_kb[:], Hsl(b, 0, cs), start=True, stop=False)
                nc.tensor.matmul(st_s[:], ones_kb[:], Hsl(b, 1, cs), start=False, stop=True)
                st_q = ps_mm.tile([1, 512], F32, name="st_q", tag="mm")
                nc.tensor.matmul(st_q[:], ones_kb[:], sq[:, 0:512], start=True, stop=False)
                nc.tensor.matmul(st_q[:], ones_kb[:], sq[:, 512:1024], start=False, stop=True)
                # xsq = (s0/64)^2 ; mu = s0/E ; xr = stq - xsq/E ;
                # rstd = 1/sqrt(xr*4096/E + eps)
                rf = rfpool.tile([1, 512], F32, name="rf", tag="rf")
                xrc = rf[:, 0:512]
                nc.scalar.activation(xrc, st_s[:], AF.Square, scale=1.0 / 64)
                nc.scalar.activation(mu_row[:, cs], st_s[:], AF.Identity, scale=1.0 / E)
                nc.vector.scalar_tensor_tensor(
                    xrc, xrc, -1.0 / E, st_q[:], op0=OP.mult, op1=OP.add
                )
                nc.scalar.activation(
                    rstd_row[:, cs], xrc, AF.Abs_reciprocal_sqrt,
                    bias=eps1[:], scale=4096.0 / E,
                )

            def ln_finish(b, g_col, be_col):
                """Broadcast rstd / mu rows back over partitions via the DRAM
                bounce on the two HW DGE queues, then the fp16 apply: one 3D
                sub + one 3D mul per chunk-pair covering both feature halves
                (plus the affine op when not FAST_LN)."""
                rstd_row = rowsB[b][:, 0:N]
                mu_row = rowsB[b][:, N : 2 * N]
                for c2 in range(CH // 2):
                    cs2 = slice(c2 * 1024, (c2 + 1) * 1024)
                    drr = drpool.tile([1, 1024], F16, name="drr", tag="drr")
                    nc.sync.dma_start(drr[:], rstd_row[:, cs2])
                    drm = drpool.tile([1, 1024], F16, name="drm", tag="drm")
                    nc.scalar.dma_start(drm[:], mu_row[:, cs2])
                    rb = bcpool.tile([P, 1024], F16, name="rb", tag="rb")
                    nc.sync.dma_start(rb[:], drr[:].to_broadcast((P, 1024)))
                    mb = bcpool.tile([P, 1024], F16, name="mb", tag="mb")
                    nc.scalar.dma_start(mb[:], drm[:].to_broadcast((P, 1024)))
                    if FAST_LN:
                        t1 = tpool.tile([P, 2048], F16, name="t1", tag="t1")
                        t13 = t1[:].rearrange("p (e n) -> p e n", e=EC)
                        nc.vector.tensor_tensor(
                            t13, H3(b, cs2), bc3(mb, 1024), op=OP.subtract
                        )
                        nc.vector.tensor_tensor(
                            H3(b, cs2), t13, bc3(rb, 1024), op=OP.mult
                        )
                    else:
                        for pt in range(EC):
                            t1 = tpool.tile([P, 1024], F16, name="t1", tag="t1")
                            nc.vector.tensor_sub(t1[:], Hsl(b, pt, cs2), mb[:])
                            t2 = tpool.tile([P, 1024], F16, name="t2", tag="t2")
                            nc.vector.tensor_mul(t2[:], t1[:], rb[:])
                            nc.vector.tensor_scalar(
                                Hsl(b, pt, cs2), t2[:],
                                g_col[:, pt : pt + 1], be_col[:, pt : pt + 1],
                                op0=OP.mult, op1=OP.add,
                            )

            def layernorm(b, g_col, be_col):
                sqs = [ln_squares(b, c) for c in range(CH)]
                for c in range(CH):
                    ln_stats_chunk(b, c, sqs[c])
                ln_finish(b, g_col, be_col)

            def input_proj(b):
                xs_t = xs_tiles[b]
                for ec in range(EC):
                    es = slice(ec * P, (ec + 1) * P)
                    for c in range(CH):
                        cs = slice(c * 512, (c + 1) * 512)
                        ps = ps_mm.tile([P, 512], F32, name="psin", tag="mm")
                        nc.tensor.matmul(ps[:], win_sb[:, es], xs_t[:, cs])
                        nc.vector.tensor_scalar_add(Hsl(b, ec, cs), ps[:], binp_sb[:, ec : ec + 1])

            def qkv(b, l):
                for w_name, dstT in (("Wk", kT[b]), ("Wq", qT[b])):
                    for dc in range(EC):
                        ds_ = slice(dc * P, (dc + 1) * P)
                        for c in range(CH):
                            cs = slice(c * 512, (c + 1) * 512)
                            ps = ps_mm.tile([P, 512], F32, name="psqk", tag="mm")
                            for ec in range(EC):
                                nc.tensor.matmul(
                                    ps[:],
                                    w_sb[w_name][l][ec][:, ds_],
                                    Hsl(b, ec, cs),
                                    start=(ec == 0),
                                    stop=(ec == EC - 1),
                                )
                            nc.scalar.copy(dstT[dc][:, cs], ps[:])
                for t in range(JT):
                    ps = ps_mm.tile([P, E], F32, name="psv", tag="mm")
                    for ec in range(EC):
                        nc.tensor.matmul(
                            ps[:],
                            Hsl(b, ec, slice(t * P, (t + 1) * P)),
                            w_sb["Wv"][l][ec][:],
                            start=(ec == 0),
                            stop=(ec == EC - 1),
                        )
                    if t % 2 == 0:
                        nc.scalar.copy(v_sb[b][:, t * E : (t + 1) * E], ps[:])
                    else:
                        nc.vector.tensor_copy(v_sb[b][:, t * E : (t + 1) * E], ps[:])

            def attention_chunk(b, c):
                cs = slice(c * 512, (c + 1) * 512)
                o_ps = [
                    ps_o.tile([P, 512], F32, name=f"o{oc}", tag="o")
                    for oc in range(EC)
                ]
                for j2 in range(JT // 2):
                    s_ps = ps_s.tile([P, 1024], F32, name="s_ps", tag="s")
                    for h in range(2):
                        j = 2 * j2 + h
                        hs = slice(h * 512, (h + 1) * 512)
                        for dc in range(EC):
                            nc.tensor.matmul(
                                s_ps[:, hs],
                                kT[b][dc][:, j * P : (j + 1) * P],
                                qT[b][dc][:, cs],
                                start=(dc == 0),
                                stop=(dc == EC - 1),
                            )
                    sr = spool.tile([P, 1024], F16, name="sr", tag="sr")
                    if RELU_ENG[j2] == "D":
                        nc.vector.tensor_relu(sr[:], s_ps[:])
                    else:
                        nc.scalar.activation(sr[:], s_ps[:], AF.Relu)
                    for h in range(2):
                        j = 2 * j2 + h
                        hs = slice(h * 512, (h + 1) * 512)
                        for oc in range(EC):
                            nc.tensor.matmul(
                                o_ps[oc][:],
                                v_sb[b][:, j * E + oc * P : j * E + (oc + 1) * P],
                                sr[:, hs],
                                start=(j == 0),
                                stop=(j == JT - 1),
                            )
                for oc in range(EC):
                    nc.vector.tensor_add(Hsl(b, oc, cs), Hsl(b, oc, cs), o_ps[oc][:])

            def attention(b):
                for c in range(CH):
                    attention_chunk(b, c)

            def mlp_chunk(b, l, c):
                cs = slice(c * 512, (c + 1) * 512)
                a_t = []
                for mc in range(EC):
                    ms = slice(mc * P, (mc + 1) * P)
                    ps = ps_mm.tile([P, 512], F32, name="psa", tag="mm")
                    for ec in range(EC):
                        nc.tensor.matmul(
                            ps[:],
                            w_sb["W1"][l][ec][:, ms],
                            Hsl(b, ec, cs),
                            start=(ec == 0),
                            stop=(ec == EC - 1),
                        )
                    a = apool.tile([P, 512], F16, name="a", tag="a")
                    nc.scalar.activation(
                        a[:], ps[:], AF.Relu, bias=bm1_sb[l][:, mc : mc + 1]
                    )
                    a_t.append(a)
                for oc in range(EC):
                    os_ = slice(oc * P, (oc + 1) * P)
                    ps = ps_mm.tile([P, 512], F32, name="psm", tag="mm")
                    for mc in range(EC):
                        nc.tensor.matmul(
                            ps[:],
                            w_sb["W2"][l][mc][:, os_],
                            a_t[mc][:],
                            start=(mc == 0),
                            stop=(mc == EC - 1),
                        )
                    nc.vector.scalar_tensor_tensor(
                        Hsl(b, oc, cs),
                        ps[:],
                        bm2_sb[l][:, oc : oc + 1],
                        Hsl(b, oc, cs),
                        op0=OP.add,
                        op1=OP.add,
                    )

            def mlp(b, l):
                for c in range(CH):
                    mlp_chunk(b, l, c)

            def readout(b):
                # prods = H (x) WoutT elementwise fp16, one 3D op per c2
                # pair; token-partials accumulate into one [1,512] PSUM row.
                prods = ropool.tile([P, 2 * N], F16, name="ros", tag="ros")
                wout3 = wout_sb[:].rearrange("p (e n) -> p e n", e=EC)
                st = ps_mm.tile([1, 512], F32, name="psro", tag="mm")
                nmm = 2 * CH
                i = 0
                for c2 in range(CH // 2):
                    cs2 = slice(c2 * 1024, (c2 + 1) * 1024)
                    prods3 = prods[:].rearrange("p (e n) -> p e n", e=EC)[:, :, cs2]
                    nc.vector.tensor_tensor(
                        prods3, H3(b, cs2), wout3[:, :, cs2], op=OP.mult
                    )
                    for cc in range(2):
                        cs = slice((2 * c2 + cc) * 512, (2 * c2 + cc + 1) * 512)
                        for ec in range(EC):
                            nc.tensor.matmul(
                                st[:], ones_kb[:],
                                prods[:, ec * N + cs.start : ec * N + cs.stop],
                                start=(i == 0), stop=(i == nmm - 1),
                            )
                            i += 1
                rsc = cpool.tile([1, 1], F32, name=f"rsc{b}", tag=f"rsc{b}")
                nc.vector.reduce_sum(rsc[:], st[:], axis=mybir.AxisListType.X)
                ob = cpool.tile([1, 1], F32, name=f"ob{b}", tag=f"ob{b}")
                nc.scalar.activation(ob[:], rsc[:], AF.Identity, bias=bout_sb[:])
                nc.sync.dma_start(d_out[b : b + 1, :], ob[:])

            # ---- software-pipelined emission ------------------------------
            input_proj(0)
            input_proj(1)
            qkv(0, 0)
            for l in range(L):
                attention(0)
                qkv(1, l)
                # LN1(b0) stats ride inside attention(b1): stats chunk c
                # after attention chunk c, so PE never waits on squares
                for c in range(CH):
                    attention_chunk(1, c)
                    ln_stats_chunk(0, c)
                ln_finish(0, g1_sb[l], be1_sb[l])
                mlp(0, l)
                layernorm(1, g1_sb[l], be1_sb[l])
                layernorm(0, g2_sb[l], be2_sb[l])
                mlp(1, l)
                if l + 1 < L:
                    qkv(0, l + 1)
                    layernorm(1, g2_sb[l], be2_sb[l])
                else:
                    readout(0)
                    layernorm(1, g2_sb[l], be2_sb[l])
                    readout(1)

    nc.compile()
    return nc


def _prep_inputs(inputs):
    f = lambda x: np.asarray(x, np.float32)
    bf = lambda x: np.ascontiguousarray(np.asarray(x, np.float32).astype(NPF16))
    xs = f(inputs["xs"])
    xsT = np.ascontiguousarray(xs.transpose(0, 2, 1)).astype(NPF16)  # [B, D, N]
    WoutT = np.ascontiguousarray(f(inputs["Wout"]).reshape(N, E).T)  # [E, N]

    def cols(v, per_l):
        v = f(v)
        if per_l:
            return np.ascontiguousarray(v.reshape(L, EC, P).transpose(0, 2, 1))
        return np.ascontiguousarray(v.reshape(EC, P).T)

    common = {
        "Win": bf(inputs["Win"]),
        "Wq": bf(inputs["Wq"]),
        "Wk": bf(inputs["Wk"]),
        "Wv": bf(inputs["Wv"]),
        "W1": bf(inputs["W1"]),
        "W2": bf(inputs["W2"]),
        "WoutT": WoutT.astype(NPF16),
        "colpack": np.concatenate(
            [cols(inputs["b_in"], False)]
            + [
                cols(inputs[k], True).transpose(1, 0, 2).reshape(P, L * EC)
                for k in ("bm1", "bm2", "be1", "be2", "g1", "g2")
            ],
            axis=1,
        ),
        "b_out": f(inputs["b_out"]).reshape(1, 1),
    }
    in_maps = []
    for c in range(NCORES):
        m = dict(common)
        m["xsT"] = np.ascontiguousarray(xsT[c * BL : (c + 1) * BL])
        in_maps.append(m)
    return in_maps


def get_program(fast_ln=True):
    key = f"nc{fast_ln}"
    if key not in _CACHE:
        _CACHE[key] = _build(FAST_LN=fast_ln)
    return _CACHE[key]


def kernel(**inputs) -> np.ndarray:
    fast = (
        np.all(np.asarray(inputs["g1"]) == 1.0)
        and np.all(np.asarray(inputs["g2"]) == 1.0)
        and np.all(np.asarray(inputs["be1"]) == 0.0)
        and np.all(np.asarray(inputs["be2"]) == 0.0)
    )
    nc = get_program(fast_ln=bool(fast))
    in_maps = _prep_inputs(inputs)
    res = run_bass_kernel_spmd(nc, in_maps, list(range(NCORES)))
    out = np.concatenate([res.results[c]["out"] for c in range(NCORES)], axis=0)
    return out.astype(np.float32)
